# revision 4
# baseline (speedup 1.0000x reference)
"""AttentionFlowLayer (BiDAF-style) Trainium2 kernel, 8 NeuronCores.

Sharding: data-parallel over batch N=16 -> 2 batches per core, weights
replicated, no collectives.

Math per batch (Lc=2048, Lq=256, D=256), per 128-row context tile:
  psum S'[i,j] = sum_d c[i,d]*w_m[d]*q[j,d] + qw[j]   (bf16 matmul, f32 psum)
  psum col 256  = cw[i] = c_i . w_c                    (extra rhs column)
  A' = exp(S' + cw_i)    (ScalarE, bias=cw)            full-bias exponential
  eb[i] = rowmax(A') = exp(max_j S[i,j])               q2c softmax numerator
  c2q psum = A' @ [q | 1] -> cols 0..255 = A'@q, col 256 = Z_i (row sum)
  c2q = (A' @ q) / Z_i   (row bias cancels in the normalized softmax)
  q2c = (sum_i eb_i * c16[i,:]) / sum_i eb_i           (matmul accumulation)
  G tile = [c, c2q, c*c2q, c*q2c] in bf16, host upcasts to f32.
"""

import numpy as np

N, LC, LQ, D = 16, 2048, 256, 256
NCORES = 8
NB = N // NCORES      # batches per core
P = 128
T = LC // P           # context tiles per batch
JT = LQ // P          # query partition tiles
DC = D // P           # d chunks
OG = 4                # output tile-group size (tiles per output DMA)

_cache = {}


def _build():
    import concourse.mybir as mybir
    from concourse import bacc
    from concourse.tile import TileContext
    from concourse.masks import make_identity

    f32 = mybir.dt.float32
    bf16 = mybir.dt.bfloat16
    EXP = mybir.ActivationFunctionType.Exp
    COPY = mybir.ActivationFunctionType.Copy
    AX = mybir.AxisListType.X

    nc = bacc.Bacc("TRN2")
    c_in = nc.dram_tensor("emb_context", (NB, LC, D), f32, kind="ExternalInput")
    q_in = nc.dram_tensor("emb_query", (NB, LQ, D), f32, kind="ExternalInput")
    w_in = nc.dram_tensor("W", (3 * D,), f32, kind="ExternalInput")
    out = nc.dram_tensor("out", (NB, LC, 4 * D), bf16, kind="ExternalOutput")

    with TileContext(nc) as tc:
        with (
            tc.tile_pool(name="const", bufs=1) as constp,
            tc.tile_pool(name="qpool", bufs=2) as qpool,
            tc.tile_pool(name="cfull", bufs=2) as cfp,
            tc.tile_pool(name="ct", bufs=4) as ctp,
            tc.tile_pool(name="apool", bufs=4) as app,
            tc.tile_pool(name="aptp", bufs=4) as aptp,
            tc.tile_pool(name="gbig", bufs=2) as gp,
            tc.tile_pool(name="small", bufs=8) as smallp,
            tc.tile_pool(name="batchp", bufs=2) as bp,
            tc.tile_pool(name="ps_s", bufs=2, space="PSUM") as ps_s,
            tc.tile_pool(name="ps_t", bufs=3, space="PSUM") as ps_t,
            tc.tile_pool(name="ps_cq", bufs=2, space="PSUM") as ps_cq,
            tc.tile_pool(name="ps_sm", bufs=1, space="PSUM") as ps_sm,
        ):
            ident = constp.tile([P, P], bf16, tag="ident")
            make_identity(nc, ident)
            ones_row = constp.tile([1, P], bf16, tag="ones_row")
            nc.vector.memset(ones_row, 1.0)
            ones_col = constp.tile([P, 1], bf16, tag="ones_col")
            nc.vector.memset(ones_col, 1.0)
            # W columns: [wc0 wc1 wq0 wq1 wm0 wm1], chunk c covers d=c*128..c*128+127
            wcols = constp.tile([P, 6], f32, tag="wcols")
            nc.scalar.dma_start(wcols, w_in[:].rearrange("(c p) -> p c", p=P))
            wq16 = constp.tile([P, 2], bf16, tag="wq16")
            nc.vector.tensor_copy(wq16, wcols[:, 2:4])

            for b in range(NB):
                # ---- per-batch input loads (single big DMAs, ACT hwdge ring)
                cfull = cfp.tile([P, T, D], f32, tag="cfull")
                nc.scalar.dma_start(cfull, c_in[b].rearrange("(t p) d -> p t d", p=P))
                qf = qpool.tile([P, JT, D], f32, tag="qf")
                nc.scalar.dma_start(qf, q_in[b].rearrange("(jt p) d -> p jt d", p=P))

                # q16x: bf16 queries with a ones column (Z accumulator)
                q16x = qpool.tile([P, JT, D + 1], bf16, tag="q16x")
                nc.vector.tensor_copy(q16x[:, :, 0:D], qf)
                nc.vector.memset(q16x[:, :, D:D + 1], 1.0)
                # qT16[p, c, j] = q16[j, c*128+p]
                qT16 = qpool.tile([P, DC, LQ], bf16, tag="qT16")
                for c in range(DC):
                    pst = ps_t.tile([P, LQ], bf16, tag="pst")
                    for jt in range(JT):
                        nc.tensor.transpose(
                            pst[:, jt * P:(jt + 1) * P],
                            q16x[:, jt, c * P:(c + 1) * P],
                            ident,
                        )
                    nc.vector.tensor_copy(qT16[:, c, :], pst)
                # qmTx[:, c, 0:LQ] = qT16 * w_m[c];  col LQ = w_c[c]
                qmTx = qpool.tile([P, DC, LQ + 1], bf16, tag="qmTx")
                for c in range(DC):
                    nc.vector.tensor_scalar_mul(
                        qmTx[:, c, 0:LQ], qT16[:, c, :], wcols[:, 4 + c:5 + c]
                    )
                    nc.vector.tensor_copy(qmTx[:, c, LQ:LQ + 1], wcols[:, c:c + 1])
                # qw row: qw[j] = q_j . w_q ; col LQ stays 0
                ps_qw = ps_sm.tile([1, LQ], f32, tag="sm")
                for c in range(DC):
                    nc.tensor.matmul(
                        ps_qw,
                        lhsT=wq16[:, c:c + 1],
                        rhs=qT16[:, c, :],
                        start=(c == 0),
                        stop=(c == DC - 1),
                    )
                qwx = qpool.tile([1, LQ + 1], bf16, tag="qwx")
                nc.vector.memset(qwx, 0.0)
                nc.vector.tensor_copy(qwx[:, 0:LQ], ps_qw)

                # per-batch staging / stats
                g012 = gp.tile([P, T, 3 * D], bf16, tag="g012")
                g3 = gp.tile([P, T, D], bf16, tag="g3")
                ebstag = bp.tile([P, T], f32, tag="ebstag")
                cwstag = bp.tile([P, T], f32, tag="cwstag")

                # ---- pass 1: per context tile ----
                for t in range(T):
                    # chunk0: c in bf16 (also feeds matmuls / muls)
                    nc.vector.tensor_copy(g012[:, t, 0:D], cfull[:, t, :])
                    # cT16[p, c*128+i'] = c16[i', c*128+p]
                    pst = ps_t.tile([P, D], bf16, tag="pst")
                    for c in range(DC):
                        nc.tensor.transpose(
                            pst[:, c * P:(c + 1) * P],
                            g012[:, t, c * P:(c + 1) * P],
                            ident,
                        )
                    cT16 = ctp.tile([P, D], bf16, tag="ct16")
                    nc.vector.tensor_copy(cT16, pst)
                    # S' psum: cols 0..255 = S + qw, col 256 = cw
                    ps_S_t = ps_s.tile([P, LQ + 1], f32, tag="ps_s")
                    for c in range(DC):
                        nc.tensor.matmul(
                            ps_S_t,
                            lhsT=cT16[:, c * P:(c + 1) * P],
                            rhs=qmTx[:, c, :],
                            start=(c == 0),
                            stop=False,
                        )
                    nc.tensor.matmul(
                        ps_S_t, lhsT=ones_row, rhs=qwx, start=False, stop=True
                    )
                    # cw column to SBUF (exp bias + q2c numerator factor)
                    nc.scalar.copy(cwstag[:, t:t + 1], ps_S_t[:, LQ:LQ + 1])
                    # A' = exp(S' + cw)
                    Ap_t = app.tile([P, LQ], bf16, tag="ap")
                    nc.scalar.activation(
                        Ap_t, ps_S_t[:, 0:LQ], EXP, bias=cwstag[:, t:t + 1]
                    )
                    # eb = rowmax(A') = exp(max_j S)
                    nc.vector.reduce_max(ebstag[:, t:t + 1], Ap_t, axis=AX)
                    # A'T
                    psa = ps_t.tile([P, LQ], bf16, tag="pst")
                    for jt in range(JT):
                        nc.tensor.transpose(
                            psa[:, jt * P:(jt + 1) * P],
                            Ap_t[:, jt * P:(jt + 1) * P],
                            ident,
                        )
                    ApT = aptp.tile([P, LQ], bf16, tag="apt")
                    nc.scalar.copy(ApT, psa)
                    # c2q_raw psum: cols 0..255 = A'@q, col 256 = Z_i
                    ps_c2q_t = ps_cq.tile([P, D + 1], f32, tag="cq")
                    for jt in range(JT):
                        nc.tensor.matmul(
                            ps_c2q_t,
                            lhsT=ApT[:, jt * P:(jt + 1) * P],
                            rhs=q16x[:, jt, :],
                            start=(jt == 0),
                            stop=(jt == JT - 1),
                        )
                    invZ = smallp.tile([P, 1], f32, tag="invz")
                    nc.vector.reciprocal(invZ, ps_c2q_t[:, D:D + 1])
                    # chunk1 = c2q (scaled copy)
                    nc.scalar.activation(
                        g012[:, t, D:2 * D], ps_c2q_t[:, 0:D], COPY, scale=invZ
                    )
                    # chunk2 = c * c2q
                    nc.gpsimd.tensor_mul(
                        g012[:, t, 2 * D:3 * D], g012[:, t, 0:D], g012[:, t, D:2 * D]
                    )
                    # group output DMA once the last tile of the group is done
                    if t % OG == OG - 1:
                        g0 = t - (OG - 1)
                        nc.sync.dma_start(
                            out[b].rearrange("(t p) d -> p t d", p=P)[
                                :, g0:t + 1, 0:3 * D
                            ],
                            g012[:, g0:t + 1, :],
                        )

                # ---- batch finalize: q2c ----
                eb16 = bp.tile([P, T], bf16, tag="eb16")
                nc.vector.tensor_copy(eb16, ebstag)
                ebrow = smallp.tile([P, 1], f32, tag="ebrow")
                nc.vector.reduce_sum(ebrow, ebstag, axis=AX)
                ebrow16 = smallp.tile([P, 1], bf16, tag="ebrow16")
                nc.vector.tensor_copy(ebrow16, ebrow)
                ps_zb = ps_sm.tile([1, 1], f32, tag="sm")
                nc.tensor.matmul(ps_zb, lhsT=ebrow16, rhs=ones_col, start=True, stop=True)
                zb = smallp.tile([1, 1], f32, tag="zb")
                nc.vector.tensor_copy(zb, ps_zb)
                inv_zb = smallp.tile([1, 1], f32, tag="invzb")
                nc.vector.reciprocal(inv_zb, zb)
                ps_q2c = ps_sm.tile([1, D], f32, tag="sm")
                for t in range(T):
                    nc.tensor.matmul(
                        ps_q2c,
                        lhsT=eb16[:, t:t + 1],
                        rhs=g012[:, t, 0:D],
                        start=(t == 0),
                        stop=(t == T - 1),
                    )
                q2cn16 = smallp.tile([1, D], bf16, tag="q2cn")
                nc.scalar.activation(q2cn16, ps_q2c, COPY, scale=inv_zb)
                # broadcast q2c to 128 partitions
                ps_bc = ps_cq.tile([P, D], f32, tag="cq")
                nc.tensor.matmul(ps_bc, lhsT=ones_row, rhs=q2cn16, start=True, stop=True)
                q2cb16 = bp.tile([P, D], bf16, tag="q2cb")
                nc.vector.tensor_copy(q2cb16, ps_bc)
                # ---- pass 2: chunk3 = c * q2c ----
                for t in range(T):
                    nc.gpsimd.tensor_mul(g3[:, t, :], g012[:, t, 0:D], q2cb16)
                nc.sync.dma_start(
                    out[b].rearrange("(t p) d -> p t d", p=P)[:, :, 3 * D:4 * D],
                    g3,
                )

    nc.compile()
    return nc


def _get_nc():
    if "nc" not in _cache:
        _cache["nc"] = _build()
    return _cache["nc"]


def run(emb_context, emb_query, W, trace=False, **kwargs):
    from concourse.bass_utils import run_bass_kernel_spmd

    nc = _get_nc()
    emb_context = np.asarray(emb_context, dtype=np.float32)
    emb_query = np.asarray(emb_query, dtype=np.float32)
    W = np.asarray(W, dtype=np.float32)
    in_maps = [
        {
            "emb_context": np.ascontiguousarray(emb_context[c * NB:(c + 1) * NB]),
            "emb_query": np.ascontiguousarray(emb_query[c * NB:(c + 1) * NB]),
            "W": W,
        }
        for c in range(NCORES)
    ]
    res = run_bass_kernel_spmd(
        nc, in_maps, core_ids=list(range(NCORES)), trace=trace, **kwargs
    )
    outs = [np.asarray(r["out"], dtype=np.float32) for r in res.results]
    return np.concatenate(outs, axis=0), res


def kernel(emb_context, emb_query, W):
    out, _ = run(emb_context, emb_query, W, trace=False)
    return out


# revision 7
# speedup vs baseline: 1.3557x; 1.3557x over previous
"""AttentionFlowLayer (BiDAF-style) Trainium2 kernel, 8 NeuronCores.

Sharding: data-parallel over batch N=16 -> 2 batches per core, weights
replicated, no collectives.

Math per batch (Lc=2048, Lq=256, D=256), per 128-row context tile:
  psum S'[i,j] = sum_d c[i,d]*w_m[d]*q[j,d] + qw[j]   (bf16 matmul, f32 psum)
  psum col 256  = cw[i] = c_i . w_c                    (extra rhs column)
  A' = exp(S' + cw_i)    (ScalarE, bias=cw)
  eb[i] = rowmax(A') = exp(max_j S[i,j])               q2c softmax numerator
  c2q psum = A' @ [q | 1] -> cols 0..255 = A'@q, col 256 = Z_i (row sum)
  c2q = (A' @ q) / Z_i
  q2c = (sum_i eb_i * c16[i,:]) / sum_i eb_i           (matmul accumulation)
  G tile = [c, c2q, c*c2q, c*q2c] in bf16, host upcasts to f32.

Emission is phase-major across the 16 context tiles of a batch so each
engine sees long runs of back-to-back ops (pipeline overlap, PE stays warm).
"""

import numpy as np

N, LC, LQ, D = 16, 2048, 256, 256
NCORES = 8
NB = N // NCORES      # batches per core
P = 128
T = LC // P           # context tiles per batch
JT = LQ // P          # query partition tiles
DC = D // P           # d chunks
OG = 8                # tiles per chunk012 output DMA

_cache = {}


def _build():
    import concourse.mybir as mybir
    from concourse import bacc
    from concourse.tile import TileContext
    from concourse.masks import make_identity

    f32 = mybir.dt.float32
    bf16 = mybir.dt.bfloat16
    EXP = mybir.ActivationFunctionType.Exp
    COPY = mybir.ActivationFunctionType.Copy
    AX = mybir.AxisListType.X

    nc = bacc.Bacc("TRN2")
    c_in = nc.dram_tensor("emb_context", (NB, LC, D), f32, kind="ExternalInput")
    q_in = nc.dram_tensor("emb_query", (NB, LQ, D), f32, kind="ExternalInput")
    w_in = nc.dram_tensor("W", (3 * D,), f32, kind="ExternalInput")
    out = nc.dram_tensor("out", (NB, LC, 4 * D), bf16, kind="ExternalOutput")

    with TileContext(nc) as tc:
        with (
            tc.tile_pool(name="const", bufs=1) as constp,
            tc.tile_pool(name="qpool", bufs=2) as qpool,
            tc.tile_pool(name="cfull", bufs=2) as cfp,
            tc.tile_pool(name="perb", bufs=2) as perb,
            tc.tile_pool(name="gbig", bufs=2) as gp,
            tc.tile_pool(name="small", bufs=8) as smallp,
            tc.tile_pool(name="ps_s", bufs=3, space="PSUM") as ps_s,
            tc.tile_pool(name="ps_t", bufs=3, space="PSUM") as ps_t,
            tc.tile_pool(name="ps_cq", bufs=2, space="PSUM") as ps_cq,
        ):
            ident = constp.tile([P, P], bf16, tag="ident")
            make_identity(nc, ident)
            ones_row = constp.tile([1, P], bf16, tag="ones_row")
            nc.vector.memset(ones_row, 1.0)
            ones_col = constp.tile([P, 1], bf16, tag="ones_col")
            nc.vector.memset(ones_col, 1.0)
            # W columns: [wc0 wc1 wq0 wq1 wm0 wm1], chunk c covers d=c*128..c*128+127
            wcols = constp.tile([P, 6], f32, tag="wcols")
            nc.sync.dma_start(wcols, w_in[:].rearrange("(c p) -> p c", p=P))
            wq16 = constp.tile([P, 2], bf16, tag="wq16")
            nc.vector.tensor_copy(wq16, wcols[:, 2:4])

            for b in range(NB):
                # ---- per-batch input loads (single big DMAs)
                cfull = cfp.tile([P, T, D], f32, tag="cfull")
                nc.sync.dma_start(cfull, c_in[b].rearrange("(t p) d -> p t d", p=P))
                qf = qpool.tile([P, JT, D], f32, tag="qf")
                nc.sync.dma_start(qf, q_in[b].rearrange("(jt p) d -> p jt d", p=P))

                # q16x: bf16 queries with a ones column (Z accumulator)
                q16x = qpool.tile([P, JT, D + 1], bf16, tag="q16x")
                nc.vector.tensor_copy(q16x[:, :, 0:D], qf)
                nc.vector.memset(q16x[:, :, D:D + 1], 1.0)
                # qT16[p, c, j] = q16[j, c*128+p]
                qT16 = qpool.tile([P, DC, LQ], bf16, tag="qT16")
                for c in range(DC):
                    pst = ps_t.tile([P, LQ], bf16, tag="pst")
                    for jt in range(JT):
                        nc.tensor.transpose(
                            pst[:, jt * P:(jt + 1) * P],
                            q16x[:, jt, c * P:(c + 1) * P],
                            ident,
                        )
                    nc.vector.tensor_copy(qT16[:, c, :], pst)
                # qmTx[:, c, 0:LQ] = qT16 * w_m[c];  col LQ = w_c[c]
                qmTx = qpool.tile([P, DC, LQ + 1], bf16, tag="qmTx")
                for c in range(DC):
                    nc.vector.tensor_scalar_mul(
                        qmTx[:, c, 0:LQ], qT16[:, c, :], wcols[:, 4 + c:5 + c]
                    )
                    nc.vector.tensor_copy(qmTx[:, c, LQ:LQ + 1], wcols[:, c:c + 1])
                # qw row: qw[j] = q_j . w_q ; col LQ stays 0
                ps_qw = ps_s.tile([1, LQ], f32, tag="ps_s")
                for c in range(DC):
                    nc.tensor.matmul(
                        ps_qw,
                        lhsT=wq16[:, c:c + 1],
                        rhs=qT16[:, c, :],
                        start=(c == 0),
                        stop=(c == DC - 1),
                    )
                qwx = qpool.tile([1, LQ + 1], bf16, tag="qwx")
                nc.vector.memset(qwx, 0.0)
                nc.vector.tensor_copy(qwx[:, 0:LQ], ps_qw)

                # per-batch staging / stats (all resident for the batch)
                g012 = gp.tile([P, T, 3 * D], bf16, tag="g012")
                g3 = gp.tile([P, T, D], bf16, tag="g3")
                ebstag = perb.tile([P, T], f32, tag="ebstag")
                cwstag = perb.tile([P, T], f32, tag="cwstag")
                cT16 = perb.tile([P, T, D], bf16, tag="ct16")
                Ap = perb.tile([P, T, LQ], bf16, tag="ap")
                ApT = perb.tile([P, T, LQ], bf16, tag="apt")
                invZ = perb.tile([P, T], f32, tag="invz")

                # ---- phase A: cast c -> bf16 (chunk0) + transpose ----
                for t in range(T):
                    nc.vector.tensor_copy(g012[:, t, 0:D], cfull[:, t, :])
                    pst = ps_t.tile([P, D], bf16, tag="pst")
                    for c in range(DC):
                        nc.tensor.transpose(
                            pst[:, c * P:(c + 1) * P],
                            g012[:, t, c * P:(c + 1) * P],
                            ident,
                        )
                    nc.vector.tensor_copy(cT16[:, t, :], pst)

                # ---- phase B: S matmuls + cw + exp ----
                for t in range(T):
                    ps_S_t = ps_s.tile([P, LQ + 1], f32, tag="ps_s")
                    for c in range(DC):
                        nc.tensor.matmul(
                            ps_S_t,
                            lhsT=cT16[:, t, c * P:(c + 1) * P],
                            rhs=qmTx[:, c, :],
                            start=(c == 0),
                            stop=False,
                        )
                    nc.tensor.matmul(
                        ps_S_t, lhsT=ones_row, rhs=qwx, start=False, stop=True
                    )
                    nc.scalar.copy(cwstag[:, t:t + 1], ps_S_t[:, LQ:LQ + 1])
                    nc.scalar.activation(
                        Ap[:, t, :], ps_S_t[:, 0:LQ], EXP, bias=cwstag[:, t:t + 1]
                    )

                # ---- phase C: rowmax + A' transpose ----
                for t in range(T):
                    nc.vector.reduce_max(ebstag[:, t:t + 1], Ap[:, t, :], axis=AX)
                    psa = ps_t.tile([P, LQ], bf16, tag="pst")
                    for jt in range(JT):
                        nc.tensor.transpose(
                            psa[:, jt * P:(jt + 1) * P],
                            Ap[:, t, jt * P:(jt + 1) * P],
                            ident,
                        )
                    if t % 2 == 0:
                        nc.scalar.copy(ApT[:, t, :], psa)
                    else:
                        nc.vector.tensor_copy(ApT[:, t, :], psa)

                # ---- phase D: c2q matmuls + normalize ----
                for t in range(T):
                    ps_c2q_t = ps_cq.tile([P, D + 1], f32, tag="cq")
                    for jt in range(JT):
                        nc.tensor.matmul(
                            ps_c2q_t,
                            lhsT=ApT[:, t, jt * P:(jt + 1) * P],
                            rhs=q16x[:, jt, :],
                            start=(jt == 0),
                            stop=(jt == JT - 1),
                        )
                    nc.vector.reciprocal(invZ[:, t:t + 1], ps_c2q_t[:, D:D + 1])
                    nc.scalar.activation(
                        g012[:, t, D:2 * D], ps_c2q_t[:, 0:D], COPY,
                        scale=invZ[:, t:t + 1],
                    )

                # ---- phase E: chunk2 = c*c2q (gpsimd) + group DMAs ----
                for t in range(T):
                    nc.gpsimd.tensor_mul(
                        g012[:, t, 2 * D:3 * D], g012[:, t, 0:D], g012[:, t, D:2 * D]
                    )
                    if t % OG == OG - 1:
                        g0 = t - (OG - 1)
                        nc.sync.dma_start(
                            out[b].rearrange("(t p) d -> p t d", p=P)[
                                :, g0:t + 1, 0:3 * D
                            ],
                            g012[:, g0:t + 1, :],
                        )

                # ---- phase F: q2c ----
                eb16 = perb.tile([P, T], bf16, tag="eb16")
                nc.vector.tensor_copy(eb16, ebstag)
                ebrow = smallp.tile([P, 1], f32, tag="ebrow")
                nc.vector.reduce_sum(ebrow, ebstag, axis=AX)
                ebrow16 = smallp.tile([P, 1], bf16, tag="ebrow16")
                nc.vector.tensor_copy(ebrow16, ebrow)
                ps_zb = ps_s.tile([1, 1], f32, tag="ps_s")
                nc.tensor.matmul(ps_zb, lhsT=ebrow16, rhs=ones_col, start=True, stop=True)
                zb = smallp.tile([1, 1], f32, tag="zb")
                nc.vector.tensor_copy(zb, ps_zb)
                inv_zb = smallp.tile([1, 1], f32, tag="invzb")
                nc.vector.reciprocal(inv_zb, zb)
                ps_q2c = ps_s.tile([1, D], f32, tag="ps_s")
                for t in range(T):
                    nc.tensor.matmul(
                        ps_q2c,
                        lhsT=eb16[:, t:t + 1],
                        rhs=g012[:, t, 0:D],
                        start=(t == 0),
                        stop=(t == T - 1),
                    )
                q2cn16 = smallp.tile([1, D], bf16, tag="q2cn")
                nc.scalar.activation(q2cn16, ps_q2c, COPY, scale=inv_zb)
                ps_bc = ps_cq.tile([P, D], f32, tag="cq")
                nc.tensor.matmul(ps_bc, lhsT=ones_row, rhs=q2cn16, start=True, stop=True)
                q2cb16 = perb.tile([P, D], bf16, tag="q2cb")
                nc.vector.tensor_copy(q2cb16, ps_bc)

                # ---- phase G: chunk3 = c * q2c (split DVE/gpsimd) + DMA ----
                for t in range(T):
                    eng = nc.vector if t % 2 == 0 else nc.gpsimd
                    eng.tensor_mul(g3[:, t, :], g012[:, t, 0:D], q2cb16)
                nc.sync.dma_start(
                    out[b].rearrange("(t p) d -> p t d", p=P)[:, :, 3 * D:4 * D],
                    g3,
                )

    nc.compile()
    return nc


def _get_nc():
    if "nc" not in _cache:
        _cache["nc"] = _build()
    return _cache["nc"]


def run(emb_context, emb_query, W, trace=False, **kwargs):
    from concourse.bass_utils import run_bass_kernel_spmd

    nc = _get_nc()
    emb_context = np.asarray(emb_context, dtype=np.float32)
    emb_query = np.asarray(emb_query, dtype=np.float32)
    W = np.asarray(W, dtype=np.float32)
    in_maps = [
        {
            "emb_context": np.ascontiguousarray(emb_context[c * NB:(c + 1) * NB]),
            "emb_query": np.ascontiguousarray(emb_query[c * NB:(c + 1) * NB]),
            "W": W,
        }
        for c in range(NCORES)
    ]
    res = run_bass_kernel_spmd(
        nc, in_maps, core_ids=list(range(NCORES)), trace=trace, **kwargs
    )
    outs = [np.asarray(r["out"], dtype=np.float32) for r in res.results]
    return np.concatenate(outs, axis=0), res


def kernel(emb_context, emb_query, W):
    out, _ = run(emb_context, emb_query, W, trace=False)
    return out


# revision 10
# speedup vs baseline: 1.6138x; 1.1904x over previous
"""AttentionFlowLayer (BiDAF-style) Trainium2 kernel, 8 NeuronCores.

Sharding: data-parallel over batch N=16 -> 2 batches per core, weights
replicated, no collectives.

Math per batch (Lc=2048, Lq=256, D=256), per 128-row context tile:
  psum S'[i,j] = sum_d c[i,d]*w_m[d]*q[j,d] + qw[j]   (bf16 matmul, f32 psum)
  psum col 256  = cw[i] = c_i . w_c                    (extra rhs column)
  A' = exp(S' + cw_i)    (ScalarE, bias=cw)
  eb[i] = rowmax(A') = exp(max_j S[i,j])               q2c softmax numerator
  c2q psum = A' @ [q | 1] -> cols 0..255 = A'@q, col 256 = Z_i (row sum)
  c2q = (A' @ q) / Z_i
  q2c = (sum_i eb_i * c16[i,:]) / sum_i eb_i           (matmul accumulation)
  G tile = [c, c2q, c*c2q, c*q2c] in bf16, host upcasts to f32.

Emission is phase-major across the 16 context tiles of a batch so each
engine sees long runs of back-to-back ops (pipeline overlap, PE stays warm).
"""

import numpy as np

N, LC, LQ, D = 16, 2048, 256, 256
NCORES = 8
NB = N // NCORES      # batches per core
P = 128
T = LC // P           # context tiles per batch
JT = LQ // P          # query partition tiles
DC = D // P           # d chunks
OG = 4                # tiles per output DMA group

_cache = {}


def _build():
    import concourse.mybir as mybir
    from concourse import bacc
    from concourse.tile import TileContext
    from concourse.masks import make_identity

    f32 = mybir.dt.float32
    bf16 = mybir.dt.bfloat16
    EXP = mybir.ActivationFunctionType.Exp
    COPY = mybir.ActivationFunctionType.Copy
    AX = mybir.AxisListType.X

    nc = bacc.Bacc("TRN2")
    c_in = nc.dram_tensor("emb_context", (NB, LC, D), f32, kind="ExternalInput")
    q_in = nc.dram_tensor("emb_query", (NB, LQ, D), f32, kind="ExternalInput")
    w_in = nc.dram_tensor("W", (3 * D,), f32, kind="ExternalInput")
    out = nc.dram_tensor("out", (NB, LC, 4 * D), bf16, kind="ExternalOutput")

    with TileContext(nc) as tc:
        with (
            tc.tile_pool(name="const", bufs=1) as constp,
            tc.tile_pool(name="qpool", bufs=2) as qpool,
            tc.tile_pool(name="cfull", bufs=2) as cfp,
            tc.tile_pool(name="perb", bufs=2) as perb,
            tc.tile_pool(name="gbig", bufs=2) as gp,
            tc.tile_pool(name="small", bufs=8) as smallp,
            tc.tile_pool(name="ps_s", bufs=3, space="PSUM") as ps_s,
            tc.tile_pool(name="ps_t", bufs=3, space="PSUM") as ps_t,
            tc.tile_pool(name="ps_cq", bufs=2, space="PSUM") as ps_cq,
        ):
            ident = constp.tile([P, P], bf16, tag="ident")
            make_identity(nc, ident)
            ones_row = constp.tile([1, P], bf16, tag="ones_row")
            nc.vector.memset(ones_row, 1.0)
            ones_col = constp.tile([P, 1], bf16, tag="ones_col")
            nc.vector.memset(ones_col, 1.0)
            # W columns: [wc0 wc1 wq0 wq1 wm0 wm1], chunk c covers d=c*128..c*128+127
            wcols = constp.tile([P, 6], f32, tag="wcols")
            nc.sync.dma_start(wcols, w_in[:].rearrange("(c p) -> p c", p=P))
            wq16 = constp.tile([P, 2], bf16, tag="wq16")
            nc.vector.tensor_copy(wq16, wcols[:, 2:4])

            for b in range(NB):
                # ---- per-batch input loads (query first, context in quarters)
                qf = qpool.tile([P, JT, D], f32, tag="qf")
                nc.sync.dma_start(qf, q_in[b].rearrange("(jt p) d -> p jt d", p=P))
                cfull = cfp.tile([P, T, D], f32, tag="cfull")
                CQ = T // 4
                c_r = c_in[b].rearrange("(t p) d -> p t d", p=P)
                for i in range(4):
                    nc.sync.dma_start(
                        cfull[:, i * CQ:(i + 1) * CQ, :], c_r[:, i * CQ:(i + 1) * CQ, :]
                    )

                # q16x: bf16 queries with a ones column (Z accumulator)
                q16x = qpool.tile([P, JT, D + 1], bf16, tag="q16x")
                nc.vector.tensor_copy(q16x[:, :, 0:D], qf)
                nc.vector.memset(q16x[:, :, D:D + 1], 1.0)
                # qT16[p, c, j] = q16[j, c*128+p]
                qT16 = qpool.tile([P, DC, LQ], bf16, tag="qT16")
                for c in range(DC):
                    pst = ps_t.tile([P, LQ], bf16, tag="pst")
                    for jt in range(JT):
                        nc.tensor.transpose(
                            pst[:, jt * P:(jt + 1) * P],
                            q16x[:, jt, c * P:(c + 1) * P],
                            ident,
                        )
                    nc.vector.tensor_copy(qT16[:, c, :], pst)
                # qmTx[:, c, 0:LQ] = qT16 * w_m[c];  col LQ = w_c[c]
                qmTx = qpool.tile([P, DC, LQ + 1], bf16, tag="qmTx")
                for c in range(DC):
                    nc.vector.tensor_scalar_mul(
                        qmTx[:, c, 0:LQ], qT16[:, c, :], wcols[:, 4 + c:5 + c]
                    )
                    nc.vector.tensor_copy(qmTx[:, c, LQ:LQ + 1], wcols[:, c:c + 1])
                # qw row: qw[j] = q_j . w_q ; col LQ stays 0
                ps_qw = ps_s.tile([1, LQ], f32, tag="ps_s")
                for c in range(DC):
                    nc.tensor.matmul(
                        ps_qw,
                        lhsT=wq16[:, c:c + 1],
                        rhs=qT16[:, c, :],
                        start=(c == 0),
                        stop=(c == DC - 1),
                    )
                qwx = qpool.tile([1, LQ + 1], bf16, tag="qwx")
                nc.vector.memset(qwx, 0.0)
                nc.vector.tensor_copy(qwx[:, 0:LQ], ps_qw)

                # per-batch staging / stats (all resident for the batch)
                g012 = gp.tile([P, T, 3 * D], bf16, tag="g012")
                g3 = gp.tile([P, T, D], bf16, tag="g3")
                ebstag = perb.tile([P, T], f32, tag="ebstag")
                cwstag = perb.tile([P, T], f32, tag="cwstag")
                cT16 = perb.tile([P, T, D], bf16, tag="ct16")
                Ap = perb.tile([P, T, LQ], bf16, tag="ap")
                ApT = perb.tile([P, T, LQ], bf16, tag="apt")
                invZ = perb.tile([P, T], f32, tag="invz")

                # ---- phase A: cast c -> bf16 (chunk0) + transpose ----
                for t in range(T):
                    nc.vector.tensor_copy(g012[:, t, 0:D], cfull[:, t, :])
                    pst = ps_t.tile([P, D], bf16, tag="pst")
                    for c in range(DC):
                        nc.tensor.transpose(
                            pst[:, c * P:(c + 1) * P],
                            g012[:, t, c * P:(c + 1) * P],
                            ident,
                        )
                    nc.vector.tensor_copy(cT16[:, t, :], pst)

                # ---- phase B: S matmuls + cw + exp ----
                for t in range(T):
                    ps_S_t = ps_s.tile([P, LQ + 1], f32, tag="ps_s")
                    for c in range(DC):
                        nc.tensor.matmul(
                            ps_S_t,
                            lhsT=cT16[:, t, c * P:(c + 1) * P],
                            rhs=qmTx[:, c, :],
                            start=(c == 0),
                            stop=False,
                        )
                    nc.tensor.matmul(
                        ps_S_t, lhsT=ones_row, rhs=qwx, start=False, stop=True
                    )
                    nc.scalar.copy(cwstag[:, t:t + 1], ps_S_t[:, LQ:LQ + 1])
                    nc.scalar.activation(
                        Ap[:, t, :], ps_S_t[:, 0:LQ], EXP, bias=cwstag[:, t:t + 1]
                    )

                # ---- phase C: rowmax + A' transpose ----
                for t in range(T):
                    nc.vector.reduce_max(ebstag[:, t:t + 1], Ap[:, t, :], axis=AX)
                    psa = ps_t.tile([P, LQ], bf16, tag="pst")
                    for jt in range(JT):
                        nc.tensor.transpose(
                            psa[:, jt * P:(jt + 1) * P],
                            Ap[:, t, jt * P:(jt + 1) * P],
                            ident,
                        )
                    if t % 2 == 0:
                        nc.scalar.copy(ApT[:, t, :], psa)
                    else:
                        nc.vector.tensor_copy(ApT[:, t, :], psa)

                # ---- phase F: q2c (needs only ebstag + chunk0; overlaps D/E) ----
                eb16 = perb.tile([P, T], bf16, tag="eb16")
                nc.vector.tensor_copy(eb16, ebstag)
                ebrow = smallp.tile([P, 1], f32, tag="ebrow")
                nc.vector.reduce_sum(ebrow, ebstag, axis=AX)
                ebrow16 = smallp.tile([P, 1], bf16, tag="ebrow16")
                nc.vector.tensor_copy(ebrow16, ebrow)
                ps_zb = ps_s.tile([1, 1], f32, tag="ps_s")
                nc.tensor.matmul(ps_zb, lhsT=ebrow16, rhs=ones_col, start=True, stop=True)
                zb = smallp.tile([1, 1], f32, tag="zb")
                nc.vector.tensor_copy(zb, ps_zb)
                inv_zb = smallp.tile([1, 1], f32, tag="invzb")
                nc.vector.reciprocal(inv_zb, zb)
                ps_q2c = ps_s.tile([1, D], f32, tag="ps_s")
                for t in range(T):
                    nc.tensor.matmul(
                        ps_q2c,
                        lhsT=eb16[:, t:t + 1],
                        rhs=g012[:, t, 0:D],
                        start=(t == 0),
                        stop=(t == T - 1),
                    )
                q2cn16 = smallp.tile([1, D], bf16, tag="q2cn")
                nc.scalar.activation(q2cn16, ps_q2c, COPY, scale=inv_zb)
                ps_bc = ps_cq.tile([P, D], f32, tag="cq")
                nc.tensor.matmul(ps_bc, lhsT=ones_row, rhs=q2cn16, start=True, stop=True)
                q2cb16 = perb.tile([P, D], bf16, tag="q2cb")
                nc.vector.tensor_copy(q2cb16, ps_bc)

                # ---- phase D: c2q matmuls + normalize ----
                for t in range(T):
                    ps_c2q_t = ps_cq.tile([P, D + 1], f32, tag="cq")
                    for jt in range(JT):
                        nc.tensor.matmul(
                            ps_c2q_t,
                            lhsT=ApT[:, t, jt * P:(jt + 1) * P],
                            rhs=q16x[:, jt, :],
                            start=(jt == 0),
                            stop=(jt == JT - 1),
                        )
                    nc.vector.reciprocal(invZ[:, t:t + 1], ps_c2q_t[:, D:D + 1])
                    nc.scalar.activation(
                        g012[:, t, D:2 * D], ps_c2q_t[:, 0:D], COPY,
                        scale=invZ[:, t:t + 1],
                    )

                # ---- phase E: chunk2 = c*c2q, chunk3 = c*q2c, group DMAs ----
                for t in range(T):
                    nc.gpsimd.tensor_mul(
                        g012[:, t, 2 * D:3 * D], g012[:, t, 0:D], g012[:, t, D:2 * D]
                    )
                    nc.vector.tensor_mul(g3[:, t, :], g012[:, t, 0:D], q2cb16)
                    if t % OG == OG - 1:
                        g0 = t - (OG - 1)
                        nc.sync.dma_start(
                            out[b].rearrange("(t p) d -> p t d", p=P)[
                                :, g0:t + 1, 0:3 * D
                            ],
                            g012[:, g0:t + 1, :],
                        )
                        nc.sync.dma_start(
                            out[b].rearrange("(t p) d -> p t d", p=P)[
                                :, g0:t + 1, 3 * D:4 * D
                            ],
                            g3[:, g0:t + 1, :],
                        )

    nc.compile()
    return nc


def _get_nc():
    if "nc" not in _cache:
        _cache["nc"] = _build()
    return _cache["nc"]


def run(emb_context, emb_query, W, trace=False, **kwargs):
    from concourse.bass_utils import run_bass_kernel_spmd

    nc = _get_nc()
    emb_context = np.asarray(emb_context, dtype=np.float32)
    emb_query = np.asarray(emb_query, dtype=np.float32)
    W = np.asarray(W, dtype=np.float32)
    in_maps = [
        {
            "emb_context": np.ascontiguousarray(emb_context[c * NB:(c + 1) * NB]),
            "emb_query": np.ascontiguousarray(emb_query[c * NB:(c + 1) * NB]),
            "W": W,
        }
        for c in range(NCORES)
    ]
    res = run_bass_kernel_spmd(
        nc, in_maps, core_ids=list(range(NCORES)), trace=trace, **kwargs
    )
    outs = [np.asarray(r["out"], dtype=np.float32) for r in res.results]
    return np.concatenate(outs, axis=0), res


def kernel(emb_context, emb_query, W):
    out, _ = run(emb_context, emb_query, W, trace=False)
    return out


# revision 11
# speedup vs baseline: 1.7361x; 1.0758x over previous
"""AttentionFlowLayer (BiDAF-style) Trainium2 kernel, 8 NeuronCores.

Sharding: data-parallel over batch N=16 -> 2 batches per core, weights
replicated, no collectives.

Math per batch (Lc=2048, Lq=256, D=256), per 128-row context tile:
  psum S'[i,j] = sum_d c[i,d]*w_m[d]*q[j,d] + qw[j]   (bf16 matmul, f32 psum)
  psum col 256  = cw[i] = c_i . w_c                    (extra rhs column)
  A' = exp(S' + cw_i)    (ScalarE, bias=cw)
  eb[i] = rowmax(A') = exp(max_j S[i,j])               q2c softmax numerator
  c2q psum = A' @ [q | 1] -> cols 0..255 = A'@q, col 256 = Z_i (row sum)
  c2q = (A' @ q) / Z_i
  q2c = (sum_i eb_i * c16[i,:]) / sum_i eb_i           (matmul accumulation)
  G tile = [c, c2q, c*c2q, c*q2c] in bf16, host upcasts to f32.

Emission is phase-major across the 16 context tiles of a batch so each
engine sees long runs of back-to-back ops.  Inputs ride the ACT hwdge
ring, outputs the SP ring, so batch 1 loads overlap batch 0 stores.
GpSimd is avoided for element-wise work (it locks SBUF ports against DVE).
"""

import numpy as np

N, LC, LQ, D = 16, 2048, 256, 256
NCORES = 8
NB = N // NCORES      # batches per core
P = 128
T = LC // P           # context tiles per batch
JT = LQ // P          # query partition tiles
DC = D // P           # d chunks
OG = 4                # tiles per output DMA group / elementwise batch

_cache = {}


def _build():
    import concourse.mybir as mybir
    from concourse import bacc
    from concourse.tile import TileContext
    from concourse.masks import make_identity

    f32 = mybir.dt.float32
    bf16 = mybir.dt.bfloat16
    EXP = mybir.ActivationFunctionType.Exp
    COPY = mybir.ActivationFunctionType.Copy
    AX = mybir.AxisListType.X

    nc = bacc.Bacc("TRN2")
    c_in = nc.dram_tensor("emb_context", (NB, LC, D), f32, kind="ExternalInput")
    q_in = nc.dram_tensor("emb_query", (NB, LQ, D), f32, kind="ExternalInput")
    w_in = nc.dram_tensor("W", (3 * D,), f32, kind="ExternalInput")
    out = nc.dram_tensor("out", (NB, LC, 4 * D), bf16, kind="ExternalOutput")

    with TileContext(nc) as tc:
        with (
            tc.tile_pool(name="const", bufs=1) as constp,
            tc.tile_pool(name="qpool", bufs=2) as qpool,
            tc.tile_pool(name="cfull", bufs=2) as cfp,
            tc.tile_pool(name="perb", bufs=2) as perb,
            tc.tile_pool(name="gbig", bufs=2) as gp,
            tc.tile_pool(name="small", bufs=8) as smallp,
            tc.tile_pool(name="ps_s", bufs=3, space="PSUM") as ps_s,
            tc.tile_pool(name="ps_t", bufs=3, space="PSUM") as ps_t,
            tc.tile_pool(name="ps_cq", bufs=2, space="PSUM") as ps_cq,
        ):
            ident = constp.tile([P, P], bf16, tag="ident")
            make_identity(nc, ident)
            ones_row = constp.tile([1, P], bf16, tag="ones_row")
            nc.vector.memset(ones_row, 1.0)
            ones_col = constp.tile([P, 1], bf16, tag="ones_col")
            nc.vector.memset(ones_col, 1.0)
            # W columns: [wc0 wc1 wq0 wq1 wm0 wm1], chunk c covers d=c*128..c*128+127
            wcols = constp.tile([P, 6], f32, tag="wcols")
            nc.scalar.dma_start(wcols, w_in[:].rearrange("(c p) -> p c", p=P))
            wq16 = constp.tile([P, 2], bf16, tag="wq16")
            nc.vector.tensor_copy(wq16, wcols[:, 2:4])

            # ---- all input loads up-front on the ACT hwdge ring ----
            qfs, cfulls = [], []
            for b in range(NB):
                qf = qpool.tile([P, JT, D], f32, tag="qf")
                nc.scalar.dma_start(qf, q_in[b].rearrange("(jt p) d -> p jt d", p=P))
                qfs.append(qf)
            for b in range(NB):
                cfull = cfp.tile([P, T, D], f32, tag="cfull")
                c_r = c_in[b].rearrange("(t p) d -> p t d", p=P)
                CQ = T // 4
                for i in range(4):
                    nc.scalar.dma_start(
                        cfull[:, i * CQ:(i + 1) * CQ, :], c_r[:, i * CQ:(i + 1) * CQ, :]
                    )
                cfulls.append(cfull)

            for b in range(NB):
                qf = qfs[b]
                cfull = cfulls[b]
                # q16x: bf16 queries with a ones column (Z accumulator)
                q16x = qpool.tile([P, JT, D + 1], bf16, tag="q16x")
                nc.vector.tensor_copy(q16x[:, :, 0:D], qf)
                nc.vector.memset(q16x[:, :, D:D + 1], 1.0)
                # qT16[p, c, j] = q16[j, c*128+p]
                qT16 = qpool.tile([P, DC, LQ], bf16, tag="qT16")
                for c in range(DC):
                    pst = ps_t.tile([P, LQ], bf16, tag="pst")
                    for jt in range(JT):
                        nc.tensor.transpose(
                            pst[:, jt * P:(jt + 1) * P],
                            q16x[:, jt, c * P:(c + 1) * P],
                            ident,
                        )
                    nc.vector.tensor_copy(qT16[:, c, :], pst)
                # qmTx[:, c, 0:LQ] = qT16 * w_m[c];  col LQ = w_c[c]
                qmTx = qpool.tile([P, DC, LQ + 1], bf16, tag="qmTx")
                for c in range(DC):
                    nc.vector.tensor_scalar_mul(
                        qmTx[:, c, 0:LQ], qT16[:, c, :], wcols[:, 4 + c:5 + c]
                    )
                    nc.vector.tensor_copy(qmTx[:, c, LQ:LQ + 1], wcols[:, c:c + 1])
                # qw row: qw[j] = q_j . w_q ; col LQ stays 0
                ps_qw = ps_s.tile([1, LQ], f32, tag="ps_s")
                for c in range(DC):
                    nc.tensor.matmul(
                        ps_qw,
                        lhsT=wq16[:, c:c + 1],
                        rhs=qT16[:, c, :],
                        start=(c == 0),
                        stop=(c == DC - 1),
                    )
                qwx = qpool.tile([1, LQ + 1], bf16, tag="qwx")
                nc.vector.memset(qwx, 0.0)
                nc.vector.tensor_copy(qwx[:, 0:LQ], ps_qw)

                # per-batch staging / stats (all resident for the batch)
                g012 = gp.tile([P, T, 3 * D], bf16, tag="g012")
                g3 = gp.tile([P, T, D], bf16, tag="g3")
                ebstag = perb.tile([P, T], f32, tag="ebstag")
                cwstag = perb.tile([P, T], f32, tag="cwstag")
                cT16 = perb.tile([P, T, D], bf16, tag="ct16")
                Ap = perb.tile([P, T, LQ], bf16, tag="ap")
                ApT = perb.tile([P, T, LQ], bf16, tag="apt")
                invZ = perb.tile([P, T], f32, tag="invz")

                # ---- phase A: cast c -> bf16 (chunk0, batched) + transposes ----
                for t0 in range(0, T, OG):
                    nc.vector.tensor_copy(
                        g012[:, t0:t0 + OG, 0:D], cfull[:, t0:t0 + OG, :]
                    )
                    for t in range(t0, t0 + OG):
                        pst = ps_t.tile([P, D], bf16, tag="pst")
                        for c in range(DC):
                            nc.tensor.transpose(
                                pst[:, c * P:(c + 1) * P],
                                g012[:, t, c * P:(c + 1) * P],
                                ident,
                            )
                        nc.vector.tensor_copy(cT16[:, t, :], pst)

                # ---- phase B: S matmuls + cw + exp ----
                for t in range(T):
                    ps_S_t = ps_s.tile([P, LQ + 1], f32, tag="ps_s")
                    for c in range(DC):
                        nc.tensor.matmul(
                            ps_S_t,
                            lhsT=cT16[:, t, c * P:(c + 1) * P],
                            rhs=qmTx[:, c, :],
                            start=(c == 0),
                            stop=False,
                        )
                    nc.tensor.matmul(
                        ps_S_t, lhsT=ones_row, rhs=qwx, start=False, stop=True
                    )
                    nc.scalar.copy(cwstag[:, t:t + 1], ps_S_t[:, LQ:LQ + 1])
                    nc.scalar.activation(
                        Ap[:, t, :], ps_S_t[:, 0:LQ], EXP, bias=cwstag[:, t:t + 1]
                    )

                # ---- phase C: rowmax (batched) + A' transpose ----
                for t0 in range(0, T, OG):
                    nc.vector.reduce_max(
                        ebstag[:, t0:t0 + OG], Ap[:, t0:t0 + OG, :], axis=AX
                    )
                    for t in range(t0, t0 + OG):
                        psa = ps_t.tile([P, LQ], bf16, tag="pst")
                        for jt in range(JT):
                            nc.tensor.transpose(
                                psa[:, jt * P:(jt + 1) * P],
                                Ap[:, t, jt * P:(jt + 1) * P],
                                ident,
                            )
                        if t % 2 == 0:
                            nc.scalar.copy(ApT[:, t, :], psa)
                        else:
                            nc.vector.tensor_copy(ApT[:, t, :], psa)

                # ---- phase F: q2c (needs only ebstag + chunk0; overlaps D/E) ----
                eb16 = perb.tile([P, T], bf16, tag="eb16")
                nc.vector.tensor_copy(eb16, ebstag)
                ebrow = smallp.tile([P, 1], f32, tag="ebrow")
                nc.vector.reduce_sum(ebrow, ebstag, axis=AX)
                ebrow16 = smallp.tile([P, 1], bf16, tag="ebrow16")
                nc.vector.tensor_copy(ebrow16, ebrow)
                ps_zb = ps_s.tile([1, 1], f32, tag="ps_s")
                nc.tensor.matmul(ps_zb, lhsT=ebrow16, rhs=ones_col, start=True, stop=True)
                zb = smallp.tile([1, 1], f32, tag="zb")
                nc.vector.tensor_copy(zb, ps_zb)
                inv_zb = smallp.tile([1, 1], f32, tag="invzb")
                nc.vector.reciprocal(inv_zb, zb)
                ps_q2c = ps_s.tile([1, D], f32, tag="ps_s")
                for t in range(T):
                    nc.tensor.matmul(
                        ps_q2c,
                        lhsT=eb16[:, t:t + 1],
                        rhs=g012[:, t, 0:D],
                        start=(t == 0),
                        stop=(t == T - 1),
                    )
                q2cn16 = smallp.tile([1, D], bf16, tag="q2cn")
                nc.scalar.activation(q2cn16, ps_q2c, COPY, scale=inv_zb)
                ps_bc = ps_cq.tile([P, D], f32, tag="cq")
                nc.tensor.matmul(ps_bc, lhsT=ones_row, rhs=q2cn16, start=True, stop=True)
                q2cb16 = perb.tile([P, D], bf16, tag="q2cb")
                nc.vector.tensor_copy(q2cb16, ps_bc)

                # ---- phase D: c2q matmuls + normalize ----
                for t in range(T):
                    ps_c2q_t = ps_cq.tile([P, D + 1], f32, tag="cq")
                    for jt in range(JT):
                        nc.tensor.matmul(
                            ps_c2q_t,
                            lhsT=ApT[:, t, jt * P:(jt + 1) * P],
                            rhs=q16x[:, jt, :],
                            start=(jt == 0),
                            stop=(jt == JT - 1),
                        )
                    nc.vector.reciprocal(invZ[:, t:t + 1], ps_c2q_t[:, D:D + 1])
                    nc.scalar.activation(
                        g012[:, t, D:2 * D], ps_c2q_t[:, 0:D], COPY,
                        scale=invZ[:, t:t + 1],
                    )

                # ---- phase E: chunk2/chunk3 muls (batched DVE) + group DMAs ----
                for t0 in range(0, T, OG):
                    nc.vector.tensor_mul(
                        g012[:, t0:t0 + OG, 2 * D:3 * D],
                        g012[:, t0:t0 + OG, 0:D],
                        g012[:, t0:t0 + OG, D:2 * D],
                    )
                    nc.vector.tensor_mul(
                        g3[:, t0:t0 + OG, :],
                        g012[:, t0:t0 + OG, 0:D],
                        q2cb16[:, None, :].to_broadcast((P, OG, D)),
                    )
                    out_r = out[b].rearrange("(t p) d -> p t d", p=P)
                    nc.sync.dma_start(
                        out_r[:, t0:t0 + OG, 0:3 * D], g012[:, t0:t0 + OG, :]
                    )
                    nc.sync.dma_start(
                        out_r[:, t0:t0 + OG, 3 * D:4 * D], g3[:, t0:t0 + OG, :]
                    )

    nc.compile()
    return nc


def _get_nc():
    if "nc" not in _cache:
        _cache["nc"] = _build()
    return _cache["nc"]


def run(emb_context, emb_query, W, trace=False, **kwargs):
    from concourse.bass_utils import run_bass_kernel_spmd

    nc = _get_nc()
    emb_context = np.asarray(emb_context, dtype=np.float32)
    emb_query = np.asarray(emb_query, dtype=np.float32)
    W = np.asarray(W, dtype=np.float32)
    in_maps = [
        {
            "emb_context": np.ascontiguousarray(emb_context[c * NB:(c + 1) * NB]),
            "emb_query": np.ascontiguousarray(emb_query[c * NB:(c + 1) * NB]),
            "W": W,
        }
        for c in range(NCORES)
    ]
    res = run_bass_kernel_spmd(
        nc, in_maps, core_ids=list(range(NCORES)), trace=trace, **kwargs
    )
    outs = [np.asarray(r["out"], dtype=np.float32) for r in res.results]
    return np.concatenate(outs, axis=0), res


def kernel(emb_context, emb_query, W):
    out, _ = run(emb_context, emb_query, W, trace=False)
    return out
